# revision 1
# baseline (speedup 1.0000x reference)
"""Trainium2 Bass kernel for nn_DecoderGravity (edge-list gravity decoder).

Computes, for each edge e with src s=idx[0,e], dst d=idx[1,e]:
    out[e] = x[d, 128] - l * log(sum_k (x[s,k]-x[d,k])^2 + 0.01)

Strategy (8 NeuronCores, 80000 edges each):
  * On device, each core repacks the node table x [50000,129] f32 into
    gatherable tables: XH (fp16 positions, 256B rows) and XM (f32 mass,
    256B rows), each split at row 32768 because dma_gather indices are
    int16. One-time cost ~40MB of DMA per core.
  * Edges are bucketed host-side (part of sharding) by (src>=32768,
    dst>=32768) into 4 buckets so every dma_gather uses half-local
    indices; buckets are padded to fixed capacities with dummy edges.
  * Steady state: per tile of 128*kc edges, three dma_gathers (src
    positions, dst positions, dst mass), fp16 subtract (DVE), fp16
    square (ScalarE), f32 reduce (DVE), then a single log/mul/add
    epilogue over the whole core's [128, 672] result.
  * r2 for random 128-dim gaussian pairs is >= ~150, so fp16 position
    precision gives ~4e-5 scale-relative output error.
"""

import numpy as np

import concourse.bass as bass
import concourse.tile as tile
from concourse import bacc, mybir
from concourse.bass_utils import run_bass_kernel_spmd

# Problem constants (hardcoded per contract).
N = 50000
D = 129
DM = 128
E = 640000
NUM_CORES = 8
P = 128
EC = E // NUM_CORES          # 80000 edges per core
HALF = 32768                 # int16-index table split point
NHI = 17280                  # hi-half rows (17232 real + pad)
EPS = 0.01

# bucket capacities in 128-edge columns: ll, lh, hl, hh
CAPC = (280, 152, 152, 88)   # 35840, 19456, 19456, 11264 edges
COLS = sum(CAPC)             # 672
KC_BIG = 16                  # columns per gather tile (2048 indices)

f32 = mybir.dt.float32
fp16 = mybir.dt.float16
i16 = mybir.dt.int16


def _tiles(cols, k):
    out = []
    while cols > 0:
        out.append(min(k, cols))
        cols -= out[-1]
    return out


def build_program(num_cores=NUM_CORES, capc=CAPC, kc=KC_BIG, bufs=2):
    cols = sum(capc)
    nc = bacc.Bacc("TRN2", target_bir_lowering=False, debug=False,
                   num_devices=num_cores)
    x_ap = nc.dram_tensor("x", [N, D], f32, kind="ExternalInput").ap()
    s16_ap = nc.dram_tensor("src16", [P, cols * 8], i16,
                            kind="ExternalInput").ap()
    d16_ap = nc.dram_tensor("dst16", [P, cols * 8], i16,
                            kind="ExternalInput").ap()
    l_ap = nc.dram_tensor("l_param", [1, 1], f32, kind="ExternalInput").ap()
    out_ap = nc.dram_tensor("out", [P, cols], f32, kind="ExternalOutput").ap()

    xh_lo = nc.dram_tensor("xh_lo", [HALF, DM], fp16).ap()
    xh_hi = nc.dram_tensor("xh_hi", [NHI, DM], fp16).ap()
    # dst table: 512B rows [x fp16(128) | mass f32 packed as 2 u16 | pad]
    xd_lo = nc.dram_tensor("xd_lo", [HALF, 256], fp16).ap()
    xd_hi = nc.dram_tensor("xd_hi", [NHI, 256], fp16).ap()

    with tile.TileContext(nc) as tc:
        with (
            tc.tile_pool(name="bld", bufs=2) as bldp,
            tc.tile_pool(name="idx", bufs=1) as idxp,
            tc.tile_pool(name="wide", bufs=1) as widep,
            tc.tile_pool(name="gsrc", bufs=bufs) as srcp,
            tc.tile_pool(name="gdst", bufs=bufs) as dstp,
            tc.tile_pool(name="sq", bufs=2) as sqp,
        ):
            # ---- table build: lo half, then hi half -------------------
            # lo: x rows [0, 32768) viewed [128, 256, 129]
            x_lo = x_ap[0:HALF].rearrange("(p c) d -> p c d", p=P)
            xh_lo_v = xh_lo[:].rearrange("(p c) d -> p c d", p=P)
            xd_lo_v = xd_lo[:].rearrange("(p c) d -> p c d", p=P)
            # hi main: x rows [32768, 49920) viewed [128, 134, 129]
            x_hi = x_ap[HALF:HALF + 128 * 134].rearrange(
                "(p c) d -> p c d", p=P)
            xh_hi_v = xh_hi[0:128 * 134].rearrange("(p c) d -> p c d", p=P)
            xd_hi_v = xd_hi[0:128 * 134].rearrange("(p c) d -> p c d", p=P)

            def build_chunk(xv, xhv, xdv, c0, ck):
                xt = bldp.tile([P, 32, D], f32, tag="bx")
                nc.sync.dma_start(xt[:, :ck, :], xv[:, c0:c0 + ck, :])
                dt = bldp.tile([P, 32, 130], fp16, tag="bd")
                nc.vector.tensor_copy(dt[:, :ck, 0:DM], xt[:, :ck, 0:DM])
                # mass f32 bits -> two u16 lanes at cols 128:130
                nc.vector.tensor_copy(
                    dt[:, :ck, DM:DM + 2].bitcast(mybir.dt.uint16),
                    xt[:, :ck, DM:D].bitcast(mybir.dt.uint16))
                nc.sync.dma_start(xhv[:, c0:c0 + ck, :], dt[:, :ck, 0:DM])
                nc.sync.dma_start(xdv[:, c0:c0 + ck, 0:130], dt[:, :ck, :])

            for c0 in range(0, 256, 32):
                build_chunk(x_lo, xh_lo_v, xd_lo_v, c0, 32)
            for c0 in range(0, 134, 32):
                build_chunk(x_hi, xh_hi_v, xd_hi_v, c0, min(32, 134 - c0))
            # hi tail: x rows [49920, 50000) -> table rows [17152, 17232)
            xt = bldp.tile([80, 1, D], f32, tag="btail")
            nc.sync.dma_start(xt[:], x_ap[49920:50000].unsqueeze(1))
            dt = bldp.tile([80, 1, 130], fp16, tag="btaild")
            nc.vector.tensor_copy(dt[:, :, 0:DM], xt[:, :, 0:DM])
            nc.vector.tensor_copy(
                dt[:, :, DM:DM + 2].bitcast(mybir.dt.uint16),
                xt[:, :, DM:D].bitcast(mybir.dt.uint16))
            nc.sync.dma_start(xh_hi[17152:17232].unsqueeze(1), dt[:, :, 0:DM])
            nc.sync.dma_start(xd_hi[17152:17232].unsqueeze(1)[:, :, 0:130],
                              dt[:])

            # ---- small setup -----------------------------------------
            s16_sb = idxp.tile([P, cols * 8], i16, tag="s16")
            d16_sb = idxp.tile([P, cols * 8], i16, tag="d16")
            nc.sync.dma_start(s16_sb[:], s16_ap[:])
            nc.sync.dma_start(d16_sb[:], d16_ap[:])

            lrow = widep.tile([1, 1], f32, tag="lrow")
            nc.sync.dma_start(lrow[:], l_ap[:])
            lbc = widep.tile([P, 1], f32, tag="lbc")
            nc.gpsimd.partition_broadcast(lbc[:], lrow[:], channels=P)
            lneg = widep.tile([P, 1], f32, tag="lneg")
            nc.vector.tensor_scalar_mul(lneg[:], lbc[:], -1.0)
            epsb = widep.tile([P, 1], f32, tag="eps")
            nc.gpsimd.memset(epsb[:], EPS)

            r2w = widep.tile([P, cols], f32, tag="r2")
            mw = widep.tile([P, cols], f32, tag="m")
            logw = widep.tile([P, cols], f32, tag="logw")
            outw = widep.tile([P, cols], f32, tag="outw")

            # ---- gather + compute loop -------------------------------
            # bucket b = (src_hi)*2 + (dst_hi); process ll first so its
            # gathers only wait on the lo tables.
            off = 0
            for b, bc in enumerate(capc):
                s_tab = xh_lo if b < 2 else xh_hi
                d_tab = xd_lo if b % 2 == 0 else xd_hi
                for ck in _tiles(bc, kc):
                    sl = slice(off, off + ck)
                    isl = slice(off * 8, (off + ck) * 8)
                    nidx = ck * P
                    src_t = srcp.tile([P, kc, DM], fp16, tag="srct")
                    nc.gpsimd.dma_gather(src_t[:, :ck, :], s_tab[:],
                                         s16_sb[:, isl], nidx, nidx, DM,
                                         single_packet=False)
                    dst_t = dstp.tile([P, kc, 256], fp16, tag="dstt")
                    nc.gpsimd.dma_gather(dst_t[:, :ck, :], d_tab[:],
                                         d16_sb[:, isl], nidx, nidx, 256,
                                         single_packet=False)
                    nc.vector.tensor_copy(
                        mw[:, sl].unsqueeze(2),
                        dst_t[:, :ck, DM:DM + 2].bitcast(f32))
                    nc.vector.tensor_tensor(
                        out=dst_t[:, :ck, 0:DM], in0=src_t[:, :ck, :],
                        in1=dst_t[:, :ck, 0:DM], op=mybir.AluOpType.subtract)
                    sq_t = sqp.tile([P, kc, DM], fp16, tag="sq")
                    nc.scalar.activation(sq_t[:, :ck, :],
                                         dst_t[:, :ck, 0:DM],
                                         mybir.ActivationFunctionType.Square)
                    nc.vector.tensor_reduce(r2w[:, sl], sq_t[:, :ck, :],
                                            axis=mybir.AxisListType.X,
                                            op=mybir.AluOpType.add)
                    off += ck

            nc.scalar.activation(logw[:], r2w[:],
                                 mybir.ActivationFunctionType.Ln,
                                 bias=epsb[:, 0:1])
            nc.vector.scalar_tensor_tensor(
                out=outw[:], in0=logw[:], scalar=lneg[:, 0:1], in1=mw[:],
                op0=mybir.AluOpType.mult, op1=mybir.AluOpType.add)
            nc.sync.dma_start(out_ap[:], outw[:])

    nc.compile()
    return nc


_compiled = {}


def _get_compiled(capc=CAPC):
    if capc not in _compiled:
        _compiled[capc] = build_program(capc=capc)
    return _compiled[capc]


def _wrap16(vals: np.ndarray, cap_edges: int) -> np.ndarray:
    """int16 index list -> [128, cap/16] wrapped+replicated layout."""
    arr = np.zeros(cap_edges, np.int16)
    arr[: len(vals)] = vals
    w = arr.reshape(cap_edges // 16, 16).T        # [16, cap/16]
    return np.tile(w, (8, 1))                     # [128, cap/16]


def make_in_maps(x, edge_label_index, l_param, capc=CAPC):
    x = np.ascontiguousarray(np.asarray(x, dtype=np.float32))
    eli = np.asarray(edge_label_index)
    l = np.asarray(l_param, dtype=np.float32).reshape(1, 1)
    src = eli[0].astype(np.int64)
    dst = eli[1].astype(np.int64)
    in_maps = []
    orders = []
    counts_all = []
    for c in range(NUM_CORES):
        sl = slice(c * EC, (c + 1) * EC)
        s, d = src[sl], dst[sl]
        b = (s >= HALF) * 2 + (d >= HALF)
        order = np.argsort(b, kind="stable")
        counts = np.bincount(b, minlength=4)
        if np.any(counts > np.array(capc) * P):
            raise OverflowError(list(counts))
        s_loc = (s - HALF * (s >= HALF)).astype(np.int16)
        d_loc = (d - HALF * (d >= HALF)).astype(np.int16)
        sw_parts, dw_parts = [], []
        pos = 0
        for bi in range(4):
            es = order[pos: pos + counts[bi]]
            pos += counts[bi]
            cap = capc[bi] * P
            sw_parts.append(_wrap16(s_loc[es], cap))
            dw_parts.append(_wrap16(d_loc[es], cap))
        in_maps.append({
            "x": x,
            "src16": np.ascontiguousarray(np.concatenate(sw_parts, axis=1)),
            "dst16": np.ascontiguousarray(np.concatenate(dw_parts, axis=1)),
            "l_param": l,
        })
        orders.append(order)
        counts_all.append(counts)
    return in_maps, orders, counts_all


def _unshard(results, orders, counts_all, capc=CAPC):
    out = np.empty(E, np.float32)
    offs = np.cumsum([0] + [c for c in capc])
    for c in range(NUM_CORES):
        dev = results[c]["out"]            # [128, cols]
        order, counts = orders[c], counts_all[c]
        core_out = np.empty(EC, np.float32)
        pos = 0
        for bi in range(4):
            cnt = counts[bi]
            vals = dev[:, offs[bi]: offs[bi] + capc[bi]].T.ravel()[:cnt]
            core_out[order[pos: pos + cnt]] = vals
            pos += cnt
        out[c * EC:(c + 1) * EC] = core_out
    return out.reshape(E, 1)


def kernel(x, edge_label_index, l_param):
    capc = CAPC
    while True:
        try:
            in_maps, orders, counts = make_in_maps(
                x, edge_label_index, l_param, capc)
            break
        except OverflowError as e:
            # grow capacities to fit (rounded up to tile granularity)
            need = [max(int(np.ceil(n / P / 8)) * 8, c)
                    for n, c in zip(e.args[0], capc)]
            capc = tuple(need)
    nc = _get_compiled(capc)
    res = run_bass_kernel_spmd(nc, in_maps, list(range(NUM_CORES)))
    return _unshard(res.results, orders, counts, capc)



# revision 4
# speedup vs baseline: 1.5866x; 1.5866x over previous
"""Trainium2 Bass kernel for nn_DecoderGravity (edge-list gravity decoder).

Computes, for each edge e with src s=idx[0,e], dst d=idx[1,e]:
    out[e] = x[d, 128] - l * log(sum_k (x[s,k]-x[d,k])^2 + 0.01)

Strategy (8 NeuronCores, 80000 edges each):
  * On device, each core repacks the node table x [50000,129] f32 into
    gatherable tables: XH (fp16 positions, 256B rows) and XM (f32 mass,
    256B rows), each split at row 32768 because dma_gather indices are
    int16. One-time cost ~40MB of DMA per core.
  * Edges are bucketed host-side (part of sharding) by (src>=32768,
    dst>=32768) into 4 buckets so every dma_gather uses half-local
    indices; buckets are padded to fixed capacities with dummy edges.
  * Steady state: per tile of 128*kc edges, three dma_gathers (src
    positions, dst positions, dst mass), fp16 subtract (DVE), fp16
    square (ScalarE), f32 reduce (DVE), then a single log/mul/add
    epilogue over the whole core's [128, 672] result.
  * r2 for random 128-dim gaussian pairs is >= ~150, so fp16 position
    precision gives ~4e-5 scale-relative output error.
"""

import numpy as np

import concourse.bass as bass
import concourse.tile as tile
from concourse import bacc, mybir
from concourse.bass_utils import run_bass_kernel_spmd

# Problem constants (hardcoded per contract).
N = 50000
D = 129
DM = 128
E = 640000
NUM_CORES = 8
P = 128
EC = E // NUM_CORES          # 80000 edges per core
HALF = 32768                 # int16-index table split point
NHI = 17280                  # hi-half rows (17232 real + pad)
EPS = 0.01

# bucket capacities in 128-edge columns: ll, lh, hl, hh
CAPC = (280, 152, 152, 88)   # 35840, 19456, 19456, 11264 edges
COLS = sum(CAPC)             # 672
KC_BIG = 16                  # columns per gather tile (2048 indices)

f32 = mybir.dt.float32
fp16 = mybir.dt.float16
i16 = mybir.dt.int16


def _tiles(cols, k):
    out = []
    while cols > 0:
        out.append(min(k, cols))
        cols -= out[-1]
    return out


def build_program(num_cores=NUM_CORES, capc=CAPC, kc=KC_BIG, bufs=2):
    cols = sum(capc)
    nc = bacc.Bacc("TRN2", target_bir_lowering=False, debug=False,
                   num_devices=num_cores, num_swdge_queues=4,
                   dynamic_dma_scratch_size=32768)
    x_ap = nc.dram_tensor("x", [N, D], f32, kind="ExternalInput").ap()
    s16_ap = nc.dram_tensor("src16", [P, cols * 8], i16,
                            kind="ExternalInput").ap()
    d16_ap = nc.dram_tensor("dst16", [P, cols * 8], i16,
                            kind="ExternalInput").ap()
    l_ap = nc.dram_tensor("l_param", [1, 1], f32, kind="ExternalInput").ap()
    out_ap = nc.dram_tensor("out", [P, cols], f32, kind="ExternalOutput").ap()

    xh_lo = nc.dram_tensor("xh_lo", [HALF, DM], fp16).ap()
    xh_hi = nc.dram_tensor("xh_hi", [NHI, DM], fp16).ap()
    # dst table: 512B rows [x fp16(128) | mass f32 packed as 2 u16 | pad]
    xd_lo = nc.dram_tensor("xd_lo", [HALF, 256], fp16).ap()
    xd_hi = nc.dram_tensor("xd_hi", [NHI, 256], fp16).ap()

    with tile.TileContext(nc) as tc:
        with (
            tc.tile_pool(name="bld", bufs=2) as bldp,
            tc.tile_pool(name="idx", bufs=1) as idxp,
            tc.tile_pool(name="wide", bufs=1) as widep,
            tc.tile_pool(name="gsrc", bufs=bufs) as srcp,
            tc.tile_pool(name="gdst", bufs=bufs) as dstp,
            tc.tile_pool(name="diff", bufs=2) as diffp,
            tc.tile_pool(name="sq", bufs=2) as sqp,
        ):
            # ---- table build: lo half, then hi half -------------------
            # lo: x rows [0, 32768) viewed [128, 256, 129]
            x_lo = x_ap[0:HALF].rearrange("(p c) d -> p c d", p=P)
            xh_lo_v = xh_lo[:].rearrange("(p c) d -> p c d", p=P)
            xd_lo_v = xd_lo[:].rearrange("(p c) d -> p c d", p=P)
            # hi main: x rows [32768, 49920) viewed [128, 134, 129]
            x_hi = x_ap[HALF:HALF + 128 * 134].rearrange(
                "(p c) d -> p c d", p=P)
            xh_hi_v = xh_hi[0:128 * 134].rearrange("(p c) d -> p c d", p=P)
            xd_hi_v = xd_hi[0:128 * 134].rearrange("(p c) d -> p c d", p=P)

            def build_chunk(xv, xhv, xdv, c0, ck):
                xt = bldp.tile([P, 32, D], f32, tag="bx")
                nc.sync.dma_start(xt[:, :ck, :], xv[:, c0:c0 + ck, :])
                dt = bldp.tile([P, 32, 130], fp16, tag="bd")
                nc.vector.tensor_copy(dt[:, :ck, 0:DM], xt[:, :ck, 0:DM])
                # mass f32 bits -> two u16 lanes at cols 128:130
                nc.vector.tensor_copy(
                    dt[:, :ck, DM:DM + 2].bitcast(mybir.dt.uint16),
                    xt[:, :ck, DM:D].bitcast(mybir.dt.uint16))
                nc.sync.dma_start(xhv[:, c0:c0 + ck, :], dt[:, :ck, 0:DM])
                nc.sync.dma_start(xdv[:, c0:c0 + ck, 0:130], dt[:, :ck, :])

            for c0 in range(0, 256, 32):
                build_chunk(x_lo, xh_lo_v, xd_lo_v, c0, 32)
            for c0 in range(0, 134, 32):
                build_chunk(x_hi, xh_hi_v, xd_hi_v, c0, min(32, 134 - c0))
            # hi tail: x rows [49920, 50000) -> table rows [17152, 17232)
            xt = bldp.tile([80, 1, D], f32, tag="btail")
            nc.sync.dma_start(xt[:], x_ap[49920:50000].unsqueeze(1))
            dt = bldp.tile([80, 1, 130], fp16, tag="btaild")
            nc.vector.tensor_copy(dt[:, :, 0:DM], xt[:, :, 0:DM])
            nc.vector.tensor_copy(
                dt[:, :, DM:DM + 2].bitcast(mybir.dt.uint16),
                xt[:, :, DM:D].bitcast(mybir.dt.uint16))
            nc.sync.dma_start(xh_hi[17152:17232].unsqueeze(1), dt[:, :, 0:DM])
            nc.sync.dma_start(xd_hi[17152:17232].unsqueeze(1)[:, :, 0:130],
                              dt[:])

            # ---- small setup -----------------------------------------
            s16_sb = idxp.tile([P, cols * 8], i16, tag="s16")
            d16_sb = idxp.tile([P, cols * 8], i16, tag="d16")
            nc.sync.dma_start(s16_sb[:], s16_ap[:])
            nc.sync.dma_start(d16_sb[:], d16_ap[:])

            lrow = widep.tile([1, 1], f32, tag="lrow")
            nc.sync.dma_start(lrow[:], l_ap[:])
            lbc = widep.tile([P, 1], f32, tag="lbc")
            nc.gpsimd.partition_broadcast(lbc[:], lrow[:], channels=P)
            lneg = widep.tile([P, 1], f32, tag="lneg")
            nc.vector.tensor_scalar_mul(lneg[:], lbc[:], -1.0)
            epsb = widep.tile([P, 1], f32, tag="eps")
            nc.gpsimd.memset(epsb[:], EPS)

            r2w = widep.tile([P, cols], f32, tag="r2")
            mw = widep.tile([P, cols], f32, tag="m")
            logw = widep.tile([P, cols], f32, tag="logw")
            outw = widep.tile([P, cols], f32, tag="outw")

            # ---- gather + compute loop -------------------------------
            # bucket b = (src_hi)*2 + (dst_hi); process ll first so its
            # gathers only wait on the lo tables.  Gathers round-robin the
            # 4 SWDGE queues so all four Q7 core pairs generate descriptors
            # in parallel (queue q runs on cores 2q / 2q+1).
            off = 0
            qn = 0
            for b, bc in enumerate(capc):
                s_tab = xh_lo if b < 2 else xh_hi
                d_tab = xd_lo if b % 2 == 0 else xd_hi
                for ck in _tiles(bc, kc):
                    sl = slice(off, off + ck)
                    isl = slice(off * 8, (off + ck) * 8)
                    nidx = ck * P
                    src_t = srcp.tile([P, kc, DM], fp16, tag="srct")
                    nc.gpsimd.dma_gather(src_t[:, :ck, :], s_tab[:],
                                         s16_sb[:, isl], nidx, nidx, DM,
                                         single_packet=False,
                                         queue_num=qn % 4)
                    dst_t = dstp.tile([P, kc, 256], fp16, tag="dstt")
                    nc.gpsimd.dma_gather(dst_t[:, :ck, :], d_tab[:],
                                         d16_sb[:, isl], nidx, nidx, 256,
                                         single_packet=False,
                                         queue_num=(qn + 1) % 4)
                    qn += 2
                    # mass extraction on ScalarE (tolerates the strided AP;
                    # keeps DVE free and avoids the GpSimd SBUF-port clash)
                    nc.scalar.activation(
                        mw[:, sl].unsqueeze(2),
                        dst_t[:, :ck, DM:DM + 2].bitcast(f32),
                        mybir.ActivationFunctionType.Copy)
                    diff_t = diffp.tile([P, kc, DM], fp16, tag="diff")
                    nc.vector.tensor_tensor(
                        out=diff_t[:, :ck, :], in0=src_t[:, :ck, :],
                        in1=dst_t[:, :ck, 0:DM], op=mybir.AluOpType.subtract)
                    sq_t = sqp.tile([P, kc, DM], fp16, tag="sq")
                    nc.scalar.activation(sq_t[:, :ck, :],
                                         diff_t[:, :ck, :],
                                         mybir.ActivationFunctionType.Square)
                    nc.vector.tensor_reduce(r2w[:, sl], sq_t[:, :ck, :],
                                            axis=mybir.AxisListType.X,
                                            op=mybir.AluOpType.add)
                    off += ck

            nc.scalar.activation(logw[:], r2w[:],
                                 mybir.ActivationFunctionType.Ln,
                                 bias=epsb[:, 0:1])
            nc.vector.scalar_tensor_tensor(
                out=outw[:], in0=logw[:], scalar=lneg[:, 0:1], in1=mw[:],
                op0=mybir.AluOpType.mult, op1=mybir.AluOpType.add)
            nc.sync.dma_start(out_ap[:], outw[:])

    nc.compile()
    return nc


_compiled = {}


def _get_compiled(capc=CAPC):
    if capc not in _compiled:
        _compiled[capc] = build_program(capc=capc)
    return _compiled[capc]


def _wrap16(vals: np.ndarray, cap_edges: int) -> np.ndarray:
    """int16 index list -> [128, cap/16] wrapped+replicated layout."""
    arr = np.zeros(cap_edges, np.int16)
    arr[: len(vals)] = vals
    w = arr.reshape(cap_edges // 16, 16).T        # [16, cap/16]
    return np.tile(w, (8, 1))                     # [128, cap/16]


def make_in_maps(x, edge_label_index, l_param, capc=CAPC):
    x = np.ascontiguousarray(np.asarray(x, dtype=np.float32))
    eli = np.asarray(edge_label_index)
    l = np.asarray(l_param, dtype=np.float32).reshape(1, 1)
    src = eli[0].astype(np.int64)
    dst = eli[1].astype(np.int64)
    in_maps = []
    orders = []
    counts_all = []
    for c in range(NUM_CORES):
        sl = slice(c * EC, (c + 1) * EC)
        s, d = src[sl], dst[sl]
        b = (s >= HALF) * 2 + (d >= HALF)
        order = np.argsort(b, kind="stable")
        counts = np.bincount(b, minlength=4)
        if np.any(counts > np.array(capc) * P):
            raise OverflowError(list(counts))
        s_loc = (s - HALF * (s >= HALF)).astype(np.int16)
        d_loc = (d - HALF * (d >= HALF)).astype(np.int16)
        sw_parts, dw_parts = [], []
        pos = 0
        for bi in range(4):
            es = order[pos: pos + counts[bi]]
            pos += counts[bi]
            cap = capc[bi] * P
            sw_parts.append(_wrap16(s_loc[es], cap))
            dw_parts.append(_wrap16(d_loc[es], cap))
        in_maps.append({
            "x": x,
            "src16": np.ascontiguousarray(np.concatenate(sw_parts, axis=1)),
            "dst16": np.ascontiguousarray(np.concatenate(dw_parts, axis=1)),
            "l_param": l,
        })
        orders.append(order)
        counts_all.append(counts)
    return in_maps, orders, counts_all


def _unshard(results, orders, counts_all, capc=CAPC):
    out = np.empty(E, np.float32)
    offs = np.cumsum([0] + [c for c in capc])
    for c in range(NUM_CORES):
        dev = results[c]["out"]            # [128, cols]
        order, counts = orders[c], counts_all[c]
        core_out = np.empty(EC, np.float32)
        pos = 0
        for bi in range(4):
            cnt = counts[bi]
            vals = dev[:, offs[bi]: offs[bi] + capc[bi]].T.ravel()[:cnt]
            core_out[order[pos: pos + cnt]] = vals
            pos += cnt
        out[c * EC:(c + 1) * EC] = core_out
    return out.reshape(E, 1)


def kernel(x, edge_label_index, l_param):
    capc = CAPC
    while True:
        try:
            in_maps, orders, counts = make_in_maps(
                x, edge_label_index, l_param, capc)
            break
        except OverflowError as e:
            # grow capacities to fit (rounded up to tile granularity)
            need = [max(int(np.ceil(n / P / 8)) * 8, c)
                    for n, c in zip(e.args[0], capc)]
            capc = tuple(need)
    nc = _get_compiled(capc)
    res = run_bass_kernel_spmd(nc, in_maps, list(range(NUM_CORES)))
    return _unshard(res.results, orders, counts, capc)



# revision 5
# speedup vs baseline: 2.0955x; 1.3208x over previous
"""Trainium2 Bass kernel for nn_DecoderGravity (edge-list gravity decoder).

Computes, for each edge e with src s=idx[0,e], dst d=idx[1,e]:
    out[e] = x[d, 128] - l * log(sum_k (x[s,k]-x[d,k])^2 + 0.01)

Strategy (8 NeuronCores, 80000 edges each):
  * On device, each core repacks the node table x [50000,129] f32 into one
    gatherable table XD8 with 256B rows: [128 x fp8e4m3 positions | f32 mass
    | 124B pad], split at row 32768 because dma_gather indices are int16.
    One-time cost ~13MB of HBM writes per core.
  * Edges are bucketed host-side (part of sharding) by (src>=32768,
    dst>=32768) into 4 buckets so every dma_gather uses half-local
    indices; buckets are padded to fixed capacities with dummy edges.
  * Steady state: per tile of 128*kc edges, two dma_gathers (src rows, dst
    rows) issued round-robin over the 4 SWDGE queues so all four Q7 core
    pairs generate descriptors in parallel; fp8 subtract -> fp16 (DVE),
    square (ScalarE), reduce to f32 (DVE), mass extract (ScalarE), then a
    single log/mul/add epilogue over the whole core's [128, 672] result.
  * fp8 positions give ~3e-3 scale-relative output error (gate is 2e-2).
"""

import numpy as np

import concourse.bass as bass
import concourse.tile as tile
from concourse import bacc, mybir
from concourse.bass_utils import run_bass_kernel_spmd

# Problem constants (hardcoded per contract).
N = 50000
D = 129
DM = 128
E = 640000
NUM_CORES = 8
P = 128
EC = E // NUM_CORES          # 80000 edges per core
HALF = 32768                 # int16-index table split point
NHI = 17280                  # hi-half rows (17232 real + pad)
EPS = 0.01
ROW = 256                    # table row bytes: 128 fp8 pos | f32 mass | pad

# bucket capacities in 128-edge columns: ll, lh, hl, hh
CAPC = (280, 152, 152, 88)   # 35840, 19456, 19456, 11264 edges
COLS = sum(CAPC)             # 672
KC_BIG = 16                  # columns per gather tile (2048 indices)

f32 = mybir.dt.float32
fp16 = mybir.dt.float16
fp8 = mybir.dt.float8e4
u8 = mybir.dt.uint8
i16 = mybir.dt.int16


def _tiles(cols, k):
    out = []
    while cols > 0:
        out.append(min(k, cols))
        cols -= out[-1]
    return out


def build_program(num_cores=NUM_CORES, capc=CAPC, kc=KC_BIG, bufs=3):
    cols = sum(capc)
    nc = bacc.Bacc("TRN2", target_bir_lowering=False, debug=False,
                   num_devices=num_cores, num_swdge_queues=4,
                   dynamic_dma_scratch_size=32768)
    x_ap = nc.dram_tensor("x", [N, D], f32, kind="ExternalInput").ap()
    s16_ap = nc.dram_tensor("src16", [P, cols * 8], i16,
                            kind="ExternalInput").ap()
    d16_ap = nc.dram_tensor("dst16", [P, cols * 8], i16,
                            kind="ExternalInput").ap()
    l_ap = nc.dram_tensor("l_param", [1, 1], f32, kind="ExternalInput").ap()
    out_ap = nc.dram_tensor("out", [P, cols], f32, kind="ExternalOutput").ap()

    xd_lo = nc.dram_tensor("xd_lo", [HALF, ROW], u8).ap()
    xd_hi = nc.dram_tensor("xd_hi", [NHI, ROW], u8).ap()

    with tile.TileContext(nc) as tc:
        with (
            tc.tile_pool(name="bld", bufs=2) as bldp,
            tc.tile_pool(name="idx", bufs=1) as idxp,
            tc.tile_pool(name="wide", bufs=1) as widep,
            tc.tile_pool(name="gsrc", bufs=bufs) as srcp,
            tc.tile_pool(name="gdst", bufs=bufs) as dstp,
            tc.tile_pool(name="diff", bufs=2) as diffp,
            tc.tile_pool(name="sq", bufs=2) as sqp,
        ):
            # ---- table build: lo half, then hi half -------------------
            # lo: x rows [0, 32768) viewed [128, 256, 129]
            x_lo = x_ap[0:HALF].rearrange("(p c) d -> p c d", p=P)
            xd_lo_v = xd_lo[:].rearrange("(p c) d -> p c d", p=P)
            # hi main: x rows [32768, 49920) viewed [128, 134, 129]
            x_hi = x_ap[HALF:HALF + 128 * 134].rearrange(
                "(p c) d -> p c d", p=P)
            xd_hi_v = xd_hi[0:128 * 134].rearrange("(p c) d -> p c d", p=P)

            def build_chunk(xv, xdv, c0, ck):
                xt = bldp.tile([P, 32, D], f32, tag="bx")
                nc.sync.dma_start(xt[:, :ck, :], xv[:, c0:c0 + ck, :])
                dt = bldp.tile([P, 32, ROW], u8, tag="bd")
                nc.vector.tensor_copy(dt[:, :ck, 0:DM].bitcast(fp8),
                                      xt[:, :ck, 0:DM])
                nc.vector.tensor_copy(dt[:, :ck, DM:DM + 4].bitcast(f32),
                                      xt[:, :ck, DM:D])
                nc.sync.dma_start(xdv[:, c0:c0 + ck, :], dt[:, :ck, :])

            for c0 in range(0, 256, 32):
                build_chunk(x_lo, xd_lo_v, c0, 32)
            for c0 in range(0, 134, 32):
                build_chunk(x_hi, xd_hi_v, c0, min(32, 134 - c0))
            # hi tail: x rows [49920, 50000) -> table rows [17152, 17232)
            xt = bldp.tile([80, 1, D], f32, tag="btail")
            nc.sync.dma_start(xt[:], x_ap[49920:50000].unsqueeze(1))
            dt = bldp.tile([80, 1, ROW], u8, tag="btaild")
            nc.vector.tensor_copy(dt[:, :, 0:DM].bitcast(fp8),
                                  xt[:, :, 0:DM])
            nc.vector.tensor_copy(dt[:, :, DM:DM + 4].bitcast(f32),
                                  xt[:, :, DM:D])
            nc.sync.dma_start(xd_hi[17152:17232].unsqueeze(1), dt[:])

            # ---- small setup -----------------------------------------
            s16_sb = idxp.tile([P, cols * 8], i16, tag="s16")
            d16_sb = idxp.tile([P, cols * 8], i16, tag="d16")
            nc.sync.dma_start(s16_sb[:], s16_ap[:])
            nc.sync.dma_start(d16_sb[:], d16_ap[:])

            lrow = widep.tile([1, 1], f32, tag="lrow")
            nc.sync.dma_start(lrow[:], l_ap[:])
            lbc = widep.tile([P, 1], f32, tag="lbc")
            nc.gpsimd.partition_broadcast(lbc[:], lrow[:], channels=P)
            lneg = widep.tile([P, 1], f32, tag="lneg")
            nc.vector.tensor_scalar_mul(lneg[:], lbc[:], -1.0)
            epsb = widep.tile([P, 1], f32, tag="eps")
            nc.gpsimd.memset(epsb[:], EPS)

            r2w = widep.tile([P, cols], f32, tag="r2")
            mw = widep.tile([P, cols], f32, tag="m")
            logw = widep.tile([P, cols], f32, tag="logw")
            outw = widep.tile([P, cols], f32, tag="outw")

            # pre-set num_idxs registers once: a fresh to_reg(int) per gather
            # emits a Pool MOVE that eats an exec-queue slot and halves the
            # achievable gather overlap (queue depth is 4).
            nidx_regs = {}
            for bc in capc:
                for ck in _tiles(bc, kc):
                    n = ck * P
                    if n not in nidx_regs:
                        nidx_regs[n] = nc.gpsimd.to_reg(n)

            # ---- gather + compute loop -------------------------------
            # bucket b = (src_hi)*2 + (dst_hi); process ll first so its
            # gathers only wait on the lo table.  Gathers round-robin the
            # 4 SWDGE queues so all four Q7 core pairs generate descriptors
            # in parallel (queue q runs on cores 2q / 2q+1).
            off = 0
            qn = 0
            for b, bc in enumerate(capc):
                s_tab = xd_lo if b < 2 else xd_hi
                d_tab = xd_lo if b % 2 == 0 else xd_hi
                for ck in _tiles(bc, kc):
                    sl = slice(off, off + ck)
                    isl = slice(off * 8, (off + ck) * 8)
                    nidx = ck * P
                    nreg = nidx_regs[nidx]
                    src_t = srcp.tile([P, kc, ROW], u8, tag="srct")
                    nc.gpsimd.dma_gather(src_t[:, :ck, :], s_tab[:],
                                         s16_sb[:, isl], nidx, nreg, ROW,
                                         single_packet=False,
                                         queue_num=qn % 4)
                    dst_t = dstp.tile([P, kc, ROW], u8, tag="dstt")
                    nc.gpsimd.dma_gather(dst_t[:, :ck, :], d_tab[:],
                                         d16_sb[:, isl], nidx, nreg, ROW,
                                         single_packet=False,
                                         queue_num=(qn + 1) % 4)
                    qn += 2
                    # mass extraction on ScalarE (tolerates the strided AP;
                    # keeps DVE free of the GpSimd SBUF-port clash)
                    nc.scalar.activation(
                        mw[:, sl].unsqueeze(2),
                        dst_t[:, :ck, DM:DM + 4].bitcast(f32),
                        mybir.ActivationFunctionType.Copy)
                    diff_t = diffp.tile([P, kc, DM], fp16, tag="diff")
                    nc.vector.tensor_tensor(
                        out=diff_t[:, :ck, :],
                        in0=src_t[:, :ck, 0:DM].bitcast(fp8),
                        in1=dst_t[:, :ck, 0:DM].bitcast(fp8),
                        op=mybir.AluOpType.subtract)
                    sq_t = sqp.tile([P, kc, DM], fp16, tag="sq")
                    nc.scalar.activation(sq_t[:, :ck, :],
                                         diff_t[:, :ck, :],
                                         mybir.ActivationFunctionType.Square)
                    nc.vector.tensor_reduce(r2w[:, sl], sq_t[:, :ck, :],
                                            axis=mybir.AxisListType.X,
                                            op=mybir.AluOpType.add)
                    off += ck

            nc.scalar.activation(logw[:], r2w[:],
                                 mybir.ActivationFunctionType.Ln,
                                 bias=epsb[:, 0:1])
            nc.vector.scalar_tensor_tensor(
                out=outw[:], in0=logw[:], scalar=lneg[:, 0:1], in1=mw[:],
                op0=mybir.AluOpType.mult, op1=mybir.AluOpType.add)
            nc.sync.dma_start(out_ap[:], outw[:])

    nc.compile()
    return nc


_compiled = {}


def _get_compiled(capc=CAPC):
    if capc not in _compiled:
        _compiled[capc] = build_program(capc=capc)
    return _compiled[capc]


def _wrap16(vals: np.ndarray, cap_edges: int) -> np.ndarray:
    """int16 index list -> [128, cap/16] wrapped+replicated layout."""
    arr = np.zeros(cap_edges, np.int16)
    arr[: len(vals)] = vals
    w = arr.reshape(cap_edges // 16, 16).T        # [16, cap/16]
    return np.tile(w, (8, 1))                     # [128, cap/16]


def make_in_maps(x, edge_label_index, l_param, capc=CAPC):
    x = np.ascontiguousarray(np.asarray(x, dtype=np.float32))
    eli = np.asarray(edge_label_index)
    l = np.asarray(l_param, dtype=np.float32).reshape(1, 1)
    src = eli[0].astype(np.int64)
    dst = eli[1].astype(np.int64)
    in_maps = []
    orders = []
    counts_all = []
    for c in range(NUM_CORES):
        sl = slice(c * EC, (c + 1) * EC)
        s, d = src[sl], dst[sl]
        b = (s >= HALF) * 2 + (d >= HALF)
        order = np.argsort(b, kind="stable")
        counts = np.bincount(b, minlength=4)
        if np.any(counts > np.array(capc) * P):
            raise OverflowError(list(counts))
        s_loc = (s - HALF * (s >= HALF)).astype(np.int16)
        d_loc = (d - HALF * (d >= HALF)).astype(np.int16)
        sw_parts, dw_parts = [], []
        pos = 0
        for bi in range(4):
            es = order[pos: pos + counts[bi]]
            pos += counts[bi]
            cap = capc[bi] * P
            sw_parts.append(_wrap16(s_loc[es], cap))
            dw_parts.append(_wrap16(d_loc[es], cap))
        in_maps.append({
            "x": x,
            "src16": np.ascontiguousarray(np.concatenate(sw_parts, axis=1)),
            "dst16": np.ascontiguousarray(np.concatenate(dw_parts, axis=1)),
            "l_param": l,
        })
        orders.append(order)
        counts_all.append(counts)
    return in_maps, orders, counts_all


def _unshard(results, orders, counts_all, capc=CAPC):
    out = np.empty(E, np.float32)
    offs = np.cumsum([0] + [c for c in capc])
    for c in range(NUM_CORES):
        dev = results[c]["out"]            # [128, cols]
        order, counts = orders[c], counts_all[c]
        core_out = np.empty(EC, np.float32)
        pos = 0
        for bi in range(4):
            cnt = counts[bi]
            vals = dev[:, offs[bi]: offs[bi] + capc[bi]].T.ravel()[:cnt]
            core_out[order[pos: pos + cnt]] = vals
            pos += cnt
        out[c * EC:(c + 1) * EC] = core_out
    return out.reshape(E, 1)


def kernel(x, edge_label_index, l_param):
    capc = CAPC
    while True:
        try:
            in_maps, orders, counts = make_in_maps(
                x, edge_label_index, l_param, capc)
            break
        except OverflowError as e:
            # grow capacities to fit (rounded up to tile granularity)
            need = [max(int(np.ceil(n / P / 8)) * 8, c)
                    for n, c in zip(e.args[0], capc)]
            capc = tuple(need)
    nc = _get_compiled(capc)
    res = run_bass_kernel_spmd(nc, in_maps, list(range(NUM_CORES)))
    return _unshard(res.results, orders, counts, capc)


# revision 7
# speedup vs baseline: 2.3889x; 1.1400x over previous
"""Trainium2 Bass kernel for nn_DecoderGravity (edge-list gravity decoder).

Computes, for each edge e with src s=idx[0,e], dst d=idx[1,e]:
    out[e] = x[d, 128] - l * log(sum_k (x[s,k]-x[d,k])^2 + 0.01)

Strategy (8 NeuronCores, 80000 edges each):
  * On device, each core repacks the node table x [50000,129] f32 into one
    gatherable table XD8 with 256B rows: [128 x fp8e4m3 positions | f32 mass
    | 124B pad], split at row 32768 because dma_gather indices are int16.
    One-time cost ~13MB of HBM writes per core.
  * Edges are bucketed host-side (part of sharding) by (src>=32768,
    dst>=32768) into 4 buckets so every dma_gather uses half-local
    indices; buckets are padded to fixed capacities with dummy edges.
  * Steady state: per tile of 128*kc edges, two dma_gathers (src rows, dst
    rows) issued round-robin over the 4 SWDGE queues so all four Q7 core
    pairs generate descriptors in parallel; fp8 subtract -> fp16 (DVE),
    square (ScalarE), reduce to f32 (DVE), mass extract (ScalarE), then a
    single log/mul/add epilogue over the whole core's [128, 672] result.
  * fp8 positions give ~3e-3 scale-relative output error (gate is 2e-2).
"""

import numpy as np

import concourse.bass as bass
import concourse.tile as tile
from concourse import bacc, mybir
from concourse.bass_utils import run_bass_kernel_spmd

# Problem constants (hardcoded per contract).
N = 50000
D = 129
DM = 128
E = 640000
NUM_CORES = 8
P = 128
EC = E // NUM_CORES          # 80000 edges per core
HALF = 32768                 # int16-index table split point
NHI = 17280                  # hi-half rows (17232 real + pad)
EPS = 0.01
ROW = 256                    # table row bytes: 128 fp8 pos | f32 mass | pad

# bucket capacities in 128-edge columns: ll, lh, hl, hh
CAPC = (280, 152, 152, 88)   # 35840, 19456, 19456, 11264 edges
COLS = sum(CAPC)             # 672
# columns per gather tile: 7*128=896 indices -> 57 descs per SDMA lane,
# under the 64-desc packet ceiling so single_packet coalescing is legal.
KC_BIG = 7

f32 = mybir.dt.float32
fp16 = mybir.dt.float16
fp8 = mybir.dt.float8e4
u8 = mybir.dt.uint8
i16 = mybir.dt.int16


def _tiles(cols, k):
    out = []
    while cols > 0:
        out.append(min(k, cols))
        cols -= out[-1]
    return out


def build_program(num_cores=NUM_CORES, capc=CAPC, kc=KC_BIG, bufs=3):
    cols = sum(capc)
    nc = bacc.Bacc("TRN2", target_bir_lowering=False, debug=False,
                   num_devices=num_cores, num_swdge_queues=4,
                   dynamic_dma_scratch_size=32768)
    x_ap = nc.dram_tensor("x", [N, D], f32, kind="ExternalInput").ap()
    s16_ap = nc.dram_tensor("src16", [P, cols * 8], i16,
                            kind="ExternalInput").ap()
    d16_ap = nc.dram_tensor("dst16", [P, cols * 8], i16,
                            kind="ExternalInput").ap()
    l_ap = nc.dram_tensor("l_param", [1, 1], f32, kind="ExternalInput").ap()
    out_ap = nc.dram_tensor("out", [P, cols], f32, kind="ExternalOutput").ap()

    xd_lo = nc.dram_tensor("xd_lo", [HALF, ROW], u8).ap()
    xd_hi = nc.dram_tensor("xd_hi", [NHI, ROW], u8).ap()

    with tile.TileContext(nc) as tc:
        with (
            tc.tile_pool(name="bld", bufs=2) as bldp,
            tc.tile_pool(name="idx", bufs=1) as idxp,
            tc.tile_pool(name="wide", bufs=1) as widep,
            tc.tile_pool(name="gsrc", bufs=bufs) as srcp,
            tc.tile_pool(name="gdst", bufs=bufs) as dstp,
            tc.tile_pool(name="diff", bufs=2) as diffp,
            tc.tile_pool(name="sq", bufs=2) as sqp,
        ):
            # ---- table build: lo half, then hi half -------------------
            # lo: x rows [0, 32768) viewed [128, 256, 129]
            x_lo = x_ap[0:HALF].rearrange("(p c) d -> p c d", p=P)
            xd_lo_v = xd_lo[:].rearrange("(p c) d -> p c d", p=P)
            # hi main: x rows [32768, 49920) viewed [128, 134, 129]
            x_hi = x_ap[HALF:HALF + 128 * 134].rearrange(
                "(p c) d -> p c d", p=P)
            xd_hi_v = xd_hi[0:128 * 134].rearrange("(p c) d -> p c d", p=P)

            def build_chunk(xv, xdv, c0, ck):
                xt = bldp.tile([P, 32, D], f32, tag="bx")
                nc.sync.dma_start(xt[:, :ck, :], xv[:, c0:c0 + ck, :])
                dt = bldp.tile([P, 32, ROW], u8, tag="bd")
                nc.vector.tensor_copy(dt[:, :ck, 0:DM].bitcast(fp8),
                                      xt[:, :ck, 0:DM])
                nc.vector.tensor_copy(dt[:, :ck, DM:DM + 4].bitcast(f32),
                                      xt[:, :ck, DM:D])
                nc.sync.dma_start(xdv[:, c0:c0 + ck, :], dt[:, :ck, :])

            for c0 in range(0, 256, 32):
                build_chunk(x_lo, xd_lo_v, c0, 32)
            for c0 in range(0, 134, 32):
                build_chunk(x_hi, xd_hi_v, c0, min(32, 134 - c0))
            # hi tail: x rows [49920, 50000) -> table rows [17152, 17232)
            xt = bldp.tile([80, 1, D], f32, tag="btail")
            nc.sync.dma_start(xt[:], x_ap[49920:50000].unsqueeze(1))
            dt = bldp.tile([80, 1, ROW], u8, tag="btaild")
            nc.vector.tensor_copy(dt[:, :, 0:DM].bitcast(fp8),
                                  xt[:, :, 0:DM])
            nc.vector.tensor_copy(dt[:, :, DM:DM + 4].bitcast(f32),
                                  xt[:, :, DM:D])
            nc.sync.dma_start(xd_hi[17152:17232].unsqueeze(1), dt[:])

            # ---- small setup -----------------------------------------
            s16_sb = idxp.tile([P, cols * 8], i16, tag="s16")
            d16_sb = idxp.tile([P, cols * 8], i16, tag="d16")
            nc.sync.dma_start(s16_sb[:], s16_ap[:])
            nc.sync.dma_start(d16_sb[:], d16_ap[:])

            lrow = widep.tile([1, 1], f32, tag="lrow")
            nc.sync.dma_start(lrow[:], l_ap[:])
            lbc = widep.tile([P, 1], f32, tag="lbc")
            nc.gpsimd.partition_broadcast(lbc[:], lrow[:], channels=P)
            lneg = widep.tile([P, 1], f32, tag="lneg")
            nc.vector.tensor_scalar_mul(lneg[:], lbc[:], -1.0)
            epsb = widep.tile([P, 1], f32, tag="eps")
            nc.gpsimd.memset(epsb[:], EPS)

            r2w = widep.tile([P, cols], f32, tag="r2")
            mw = widep.tile([P, cols], f32, tag="m")
            logw = widep.tile([P, cols], f32, tag="logw")
            outw = widep.tile([P, cols], f32, tag="outw")

            # pre-set num_idxs registers once: a fresh to_reg(int) per gather
            # emits a Pool MOVE that eats an exec-queue slot and halves the
            # achievable gather overlap (queue depth is 4).
            nidx_regs = {}
            for bc in capc:
                for ck in _tiles(bc, kc):
                    n = ck * P
                    if n not in nidx_regs:
                        nidx_regs[n] = nc.gpsimd.to_reg(n)

            # ---- gather + compute loop -------------------------------
            # bucket b = (src_hi)*2 + (dst_hi); process ll first so its
            # gathers only wait on the lo table.  Gathers round-robin the
            # 4 SWDGE queues so all four Q7 core pairs generate descriptors
            # in parallel (queue q runs on cores 2q / 2q+1).
            off = 0
            qn = 0
            for b, bc in enumerate(capc):
                s_tab = xd_lo if b < 2 else xd_hi
                d_tab = xd_lo if b % 2 == 0 else xd_hi
                for ck in _tiles(bc, kc):
                    sl = slice(off, off + ck)
                    isl = slice(off * 8, (off + ck) * 8)
                    nidx = ck * P
                    nreg = nidx_regs[nidx]
                    src_t = srcp.tile([P, kc, ROW], u8, tag="srct")
                    nc.gpsimd.dma_gather(src_t[:, :ck, :], s_tab[:],
                                         s16_sb[:, isl], nidx, nreg, ROW,
                                         single_packet=True,
                                         queue_num=qn % 4)
                    dst_t = dstp.tile([P, kc, ROW], u8, tag="dstt")
                    nc.gpsimd.dma_gather(dst_t[:, :ck, :], d_tab[:],
                                         d16_sb[:, isl], nidx, nreg, ROW,
                                         single_packet=True,
                                         queue_num=(qn + 1) % 4)
                    qn += 2
                    # mass extraction on ScalarE (tolerates the strided AP;
                    # keeps DVE free of the GpSimd SBUF-port clash)
                    nc.scalar.activation(
                        mw[:, sl].unsqueeze(2),
                        dst_t[:, :ck, DM:DM + 4].bitcast(f32),
                        mybir.ActivationFunctionType.Copy)
                    diff_t = diffp.tile([P, kc, DM], fp16, tag="diff")
                    nc.vector.tensor_tensor(
                        out=diff_t[:, :ck, :],
                        in0=src_t[:, :ck, 0:DM].bitcast(fp8),
                        in1=dst_t[:, :ck, 0:DM].bitcast(fp8),
                        op=mybir.AluOpType.subtract)
                    sq_t = sqp.tile([P, kc, DM], fp16, tag="sq")
                    nc.scalar.activation(sq_t[:, :ck, :],
                                         diff_t[:, :ck, :],
                                         mybir.ActivationFunctionType.Square)
                    nc.vector.tensor_reduce(r2w[:, sl], sq_t[:, :ck, :],
                                            axis=mybir.AxisListType.X,
                                            op=mybir.AluOpType.add)
                    off += ck

            nc.scalar.activation(logw[:], r2w[:],
                                 mybir.ActivationFunctionType.Ln,
                                 bias=epsb[:, 0:1])
            nc.vector.scalar_tensor_tensor(
                out=outw[:], in0=logw[:], scalar=lneg[:, 0:1], in1=mw[:],
                op0=mybir.AluOpType.mult, op1=mybir.AluOpType.add)
            nc.sync.dma_start(out_ap[:], outw[:])

    nc.compile()
    return nc


_compiled = {}


def _get_compiled(capc=CAPC):
    if capc not in _compiled:
        _compiled[capc] = build_program(capc=capc)
    return _compiled[capc]


def _wrap16(vals: np.ndarray, cap_edges: int) -> np.ndarray:
    """int16 index list -> [128, cap/16] wrapped+replicated layout."""
    arr = np.zeros(cap_edges, np.int16)
    arr[: len(vals)] = vals
    w = arr.reshape(cap_edges // 16, 16).T        # [16, cap/16]
    return np.tile(w, (8, 1))                     # [128, cap/16]


def make_in_maps(x, edge_label_index, l_param, capc=CAPC):
    x = np.ascontiguousarray(np.asarray(x, dtype=np.float32))
    eli = np.asarray(edge_label_index)
    l = np.asarray(l_param, dtype=np.float32).reshape(1, 1)
    src = eli[0].astype(np.int64)
    dst = eli[1].astype(np.int64)
    in_maps = []
    orders = []
    counts_all = []
    for c in range(NUM_CORES):
        sl = slice(c * EC, (c + 1) * EC)
        s, d = src[sl], dst[sl]
        b = (s >= HALF) * 2 + (d >= HALF)
        order = np.argsort(b, kind="stable")
        counts = np.bincount(b, minlength=4)
        if np.any(counts > np.array(capc) * P):
            raise OverflowError(list(counts))
        s_loc = (s - HALF * (s >= HALF)).astype(np.int16)
        d_loc = (d - HALF * (d >= HALF)).astype(np.int16)
        sw_parts, dw_parts = [], []
        pos = 0
        for bi in range(4):
            es = order[pos: pos + counts[bi]]
            pos += counts[bi]
            cap = capc[bi] * P
            sw_parts.append(_wrap16(s_loc[es], cap))
            dw_parts.append(_wrap16(d_loc[es], cap))
        in_maps.append({
            "x": x,
            "src16": np.ascontiguousarray(np.concatenate(sw_parts, axis=1)),
            "dst16": np.ascontiguousarray(np.concatenate(dw_parts, axis=1)),
            "l_param": l,
        })
        orders.append(order)
        counts_all.append(counts)
    return in_maps, orders, counts_all


def _unshard(results, orders, counts_all, capc=CAPC):
    out = np.empty(E, np.float32)
    offs = np.cumsum([0] + [c for c in capc])
    for c in range(NUM_CORES):
        dev = results[c]["out"]            # [128, cols]
        order, counts = orders[c], counts_all[c]
        core_out = np.empty(EC, np.float32)
        pos = 0
        for bi in range(4):
            cnt = counts[bi]
            vals = dev[:, offs[bi]: offs[bi] + capc[bi]].T.ravel()[:cnt]
            core_out[order[pos: pos + cnt]] = vals
            pos += cnt
        out[c * EC:(c + 1) * EC] = core_out
    return out.reshape(E, 1)


def kernel(x, edge_label_index, l_param):
    capc = CAPC
    while True:
        try:
            in_maps, orders, counts = make_in_maps(
                x, edge_label_index, l_param, capc)
            break
        except OverflowError as e:
            # grow capacities to fit (rounded up to tile granularity)
            need = [max(int(np.ceil(n / P / 8)) * 8, c)
                    for n, c in zip(e.args[0], capc)]
            capc = tuple(need)
    nc = _get_compiled(capc)
    res = run_bass_kernel_spmd(nc, in_maps, list(range(NUM_CORES)))
    return _unshard(res.results, orders, counts, capc)


# revision 8
# speedup vs baseline: 2.8227x; 1.1816x over previous
"""Trainium2 Bass kernel for nn_DecoderGravity (edge-list gravity decoder).

Computes, for each edge e with src s=idx[0,e], dst d=idx[1,e]:
    out[e] = x[d, 128] - l * log(sum_k (x[s,k]-x[d,k])^2 + 0.01)

Strategy (8 NeuronCores, 80000 edges each):
  * On device, each core repacks the node table x [50000,129] f32 into a
    gatherable table with 256B rows: [128 x fp8e4m3 positions | f32 mass |
    124B pad], built as four row-quarters of 16384 (dma_gather indices are
    int16).  Edges are bucketed host-side by (src quarter, dst quarter)
    into 16 buckets, processed in wavefront order so bucket (i,j) only
    waits on quarters <= max(i,j): gathers start after ~1/4 of the build.
  * Steady state: per tile of 128*7 edges, two dma_gathers (src rows, dst
    rows) round-robin the 4 SWDGE queues so all four Q7 core pairs
    generate descriptors in parallel.  kc=7 keeps each SDMA lane at 57
    descriptors so single_packet coalescing stays under the 64-desc packet
    ceiling (coalescing removes the ~270ns-per-packet HBM-latency stall).
  * Compute per tile: fp8 subtract -> fp16 (DVE), square (ScalarE),
    reduce to f32 (DVE), mass extract (ScalarE).  Epilogue (Ln, l*log+m,
    output DMA) runs per bucket so the kernel tail is one tiny bucket.
  * fp8 positions give ~3e-3 scale-relative output error (gate is 2e-2).
"""

import numpy as np

import concourse.bass as bass
import concourse.tile as tile
from concourse import bacc, mybir
from concourse.bass_utils import run_bass_kernel_spmd

# Problem constants (hardcoded per contract).
N = 50000
D = 129
DM = 128
E = 640000
NUM_CORES = 8
P = 128
EC = E // NUM_CORES          # 80000 edges per core
EPS = 0.01
ROW = 256                    # table row bytes: 128 fp8 pos | f32 mass | pad

Q = 16384                    # table quarter size (int16-index reach)
QS = (0, Q, 2 * Q, 3 * Q)    # quarter row starts
QSIZE = (Q, Q, Q, 896)       # table rows per quarter (q3: 848 real + pad)

# bucket (src_quarter, dst_quarter) processing order: wavefront by
# max(si, dj) so bucket (i, j) only depends on built quarters <= level.
BUCKET_ORDER = tuple(sorted(
    ((i, j) for i in range(4) for j in range(4)),
    key=lambda t: (max(t), t)))
ORDER_POS = {b: k for k, b in enumerate(BUCKET_ORDER)}

# columns per gather tile: 7*128=896 indices -> 57 descs per SDMA lane,
# under the 64-desc packet ceiling so single_packet coalescing is legal.
KC_BIG = 7

f32 = mybir.dt.float32
fp16 = mybir.dt.float16
fp8 = mybir.dt.float8e4
u8 = mybir.dt.uint8
i16 = mybir.dt.int16


def default_capc():
    """Per-bucket capacities in 128-edge columns, ~4.5 sigma headroom."""
    frac = [Q / N, Q / N, Q / N, (N - 3 * Q) / N]
    caps = []
    for (i, j) in BUCKET_ORDER:
        m = EC * frac[i] * frac[j]
        caps.append(max(1, int(np.ceil((m + 4.5 * np.sqrt(m) + 1) / P))))
    return tuple(caps)


CAPC = default_capc()


def _tiles(cols, k):
    out = []
    while cols > 0:
        out.append(min(k, cols))
        cols -= out[-1]
    return out


def build_program(num_cores=NUM_CORES, capc=CAPC, kc=KC_BIG, bufs=8):
    cols = sum(capc)
    nc = bacc.Bacc("TRN2", target_bir_lowering=False, debug=False,
                   num_devices=num_cores, num_swdge_queues=4,
                   dynamic_dma_scratch_size=32768)
    x_ap = nc.dram_tensor("x", [N, D], f32, kind="ExternalInput").ap()
    s16_ap = nc.dram_tensor("src16", [P, cols * 8], i16,
                            kind="ExternalInput").ap()
    d16_ap = nc.dram_tensor("dst16", [P, cols * 8], i16,
                            kind="ExternalInput").ap()
    l_ap = nc.dram_tensor("l_param", [1, 1], f32, kind="ExternalInput").ap()
    out_ap = nc.dram_tensor("out", [P, cols], f32, kind="ExternalOutput").ap()

    # one DRAM tensor per quarter so Tile tracks build->gather deps at
    # quarter granularity (a single tensor would serialize every bucket
    # behind the full build).
    xq = [nc.dram_tensor(f"xq{q}", [QSIZE[q], ROW], u8).ap()
          for q in range(4)]

    with tile.TileContext(nc) as tc:
        with (
            tc.tile_pool(name="bld", bufs=2) as bldp,
            tc.tile_pool(name="idx", bufs=1) as idxp,
            tc.tile_pool(name="wide", bufs=1) as widep,
            tc.tile_pool(name="gsrc", bufs=bufs) as srcp,
            tc.tile_pool(name="gdst", bufs=bufs) as dstp,
            tc.tile_pool(name="diff", bufs=4) as diffp,
            tc.tile_pool(name="sq", bufs=4) as sqp,
        ):
            # ---- small setup (first so gathers never wait on it) ------
            s16_sb = idxp.tile([P, cols * 8], i16, tag="s16")
            d16_sb = idxp.tile([P, cols * 8], i16, tag="d16")
            nc.sync.dma_start(s16_sb[:], s16_ap[:])
            nc.sync.dma_start(d16_sb[:], d16_ap[:])

            lrow = widep.tile([1, 1], f32, tag="lrow")
            nc.sync.dma_start(lrow[:], l_ap[:])
            lbc = widep.tile([P, 1], f32, tag="lbc")
            nc.gpsimd.partition_broadcast(lbc[:], lrow[:], channels=P)
            lneg = widep.tile([P, 1], f32, tag="lneg")
            nc.vector.tensor_scalar_mul(lneg[:], lbc[:], -1.0)
            epsb = widep.tile([P, 1], f32, tag="eps")
            nc.gpsimd.memset(epsb[:], EPS)

            r2w = widep.tile([P, cols], f32, tag="r2")
            mw = widep.tile([P, cols], f32, tag="m")
            logw = widep.tile([P, cols], f32, tag="logw")
            outw = widep.tile([P, cols], f32, tag="outw")

            # pre-set num_idxs registers once: a fresh to_reg(int) per gather
            # emits a Pool MOVE that eats an exec-queue slot and halves the
            # achievable gather overlap (queue depth is 4).
            nidx_regs = {}
            for bc in capc:
                for ck in _tiles(bc, kc):
                    n = ck * P
                    if n not in nidx_regs:
                        nidx_regs[n] = nc.gpsimd.to_reg(n)

            # ---- table build, one quarter at a time -------------------
            def conv_chunk(xt, dt, ck):
                nc.vector.tensor_copy(dt[:, :ck, 0:DM].bitcast(fp8),
                                      xt[:, :ck, 0:DM])
                nc.vector.tensor_copy(dt[:, :ck, DM:DM + 4].bitcast(f32),
                                      xt[:, :ck, DM:D])

            def emit_build(q):
                if q < 3:
                    xv = x_ap[QS[q]:QS[q] + Q].rearrange(
                        "(p c) d -> p c d", p=P)
                    tv = xq[q][0:Q].rearrange("(p c) d -> p c d", p=P)
                    for c0 in range(0, Q // P, 32):
                        xt = bldp.tile([P, 32, D], f32, tag="bx")
                        nc.sync.dma_start(xt[:], xv[:, c0:c0 + 32, :])
                        dt = bldp.tile([P, 32, ROW], u8, tag="bd")
                        conv_chunk(xt, dt, 32)
                        nc.sync.dma_start(tv[:, c0:c0 + 32, :], dt[:])
                else:
                    # rows 49152..49920 as [128, 6, D], then 80-row tail
                    xv = x_ap[QS[3]:QS[3] + 768].rearrange(
                        "(p c) d -> p c d", p=P)
                    tv = xq[3][0:768].rearrange("(p c) d -> p c d", p=P)
                    xt = bldp.tile([P, 6, D], f32, tag="bx3")
                    nc.sync.dma_start(xt[:], xv[:])
                    dt = bldp.tile([P, 6, ROW], u8, tag="bd3")
                    conv_chunk(xt, dt, 6)
                    nc.sync.dma_start(tv[:], dt[:])
                    xt2 = bldp.tile([80, 1, D], f32, tag="btail")
                    nc.sync.dma_start(xt2[:], x_ap[49920:50000].unsqueeze(1))
                    dt2 = bldp.tile([80, 1, ROW], u8, tag="btaild")
                    conv_chunk(xt2, dt2, 1)
                    nc.sync.dma_start(xq[3][768:848].unsqueeze(1), dt2[:])

            # ---- gather + compute for one bucket ----------------------
            boff = [0]
            qn = [0]

            def emit_bucket(bi):
                si, dj = BUCKET_ORDER[bi]
                bc = capc[bi]
                b0 = boff[0]
                off = b0
                for ck in _tiles(bc, kc):
                    sl = slice(off, off + ck)
                    isl = slice(off * 8, (off + ck) * 8)
                    nidx = ck * P
                    nreg = nidx_regs[nidx]
                    src_t = srcp.tile([P, kc, ROW], u8, tag="srct")
                    nc.gpsimd.dma_gather(src_t[:, :ck, :], xq[si][:],
                                         s16_sb[:, isl], nidx, nreg, ROW,
                                         single_packet=True,
                                         queue_num=qn[0] % 4)
                    dst_t = dstp.tile([P, kc, ROW], u8, tag="dstt")
                    nc.gpsimd.dma_gather(dst_t[:, :ck, :], xq[dj][:],
                                         d16_sb[:, isl], nidx, nreg, ROW,
                                         single_packet=True,
                                         queue_num=(qn[0] + 1) % 4)
                    qn[0] += 2
                    # mass extraction on ScalarE (tolerates the strided AP;
                    # keeps DVE free of the GpSimd SBUF-port clash)
                    nc.scalar.activation(
                        mw[:, sl].unsqueeze(2),
                        dst_t[:, :ck, DM:DM + 4].bitcast(f32),
                        mybir.ActivationFunctionType.Copy)
                    diff_t = diffp.tile([P, kc, DM], fp16, tag="diff")
                    nc.vector.tensor_tensor(
                        out=diff_t[:, :ck, :],
                        in0=src_t[:, :ck, 0:DM].bitcast(fp8),
                        in1=dst_t[:, :ck, 0:DM].bitcast(fp8),
                        op=mybir.AluOpType.subtract)
                    sq_t = sqp.tile([P, kc, DM], fp16, tag="sq")
                    nc.scalar.activation(sq_t[:, :ck, :],
                                         diff_t[:, :ck, :],
                                         mybir.ActivationFunctionType.Square)
                    nc.vector.tensor_reduce(r2w[:, sl], sq_t[:, :ck, :],
                                            axis=mybir.AxisListType.X,
                                            op=mybir.AluOpType.add)
                    off += ck
                # per-bucket epilogue + output store
                bsl = slice(b0, b0 + bc)
                nc.scalar.activation(logw[:, bsl], r2w[:, bsl],
                                     mybir.ActivationFunctionType.Ln,
                                     bias=epsb[:, 0:1])
                nc.vector.scalar_tensor_tensor(
                    out=outw[:, bsl], in0=logw[:, bsl], scalar=lneg[:, 0:1],
                    in1=mw[:, bsl], op0=mybir.AluOpType.mult,
                    op1=mybir.AluOpType.add)
                nc.sync.dma_start(out_ap[:, bsl], outw[:, bsl])
                boff[0] += bc

            # wavefront: build quarter L, then all buckets of level L
            bi = 0
            for level in range(4):
                emit_build(level)
                while bi < 16 and max(BUCKET_ORDER[bi]) == level:
                    emit_bucket(bi)
                    bi += 1

    nc.compile()
    return nc


_compiled = {}


def _get_compiled(capc=CAPC):
    if capc not in _compiled:
        _compiled[capc] = build_program(capc=capc)
    return _compiled[capc]


def _wrap16(vals: np.ndarray, cap_edges: int) -> np.ndarray:
    """int16 index list -> [128, cap/16] wrapped+replicated layout."""
    arr = np.zeros(cap_edges, np.int16)
    arr[: len(vals)] = vals
    w = arr.reshape(cap_edges // 16, 16).T        # [16, cap/16]
    return np.tile(w, (8, 1))                     # [128, cap/16]


def make_in_maps(x, edge_label_index, l_param, capc=CAPC):
    x = np.ascontiguousarray(np.asarray(x, dtype=np.float32))
    eli = np.asarray(edge_label_index)
    l = np.asarray(l_param, dtype=np.float32).reshape(1, 1)
    src = eli[0].astype(np.int64)
    dst = eli[1].astype(np.int64)
    pos_of = np.empty(16, np.int64)
    for k, (i, j) in enumerate(BUCKET_ORDER):
        pos_of[i * 4 + j] = k
    in_maps = []
    orders = []
    counts_all = []
    for c in range(NUM_CORES):
        sl = slice(c * EC, (c + 1) * EC)
        s, d = src[sl], dst[sl]
        b = pos_of[(s >> 14) * 4 + (d >> 14)]
        order = np.argsort(b, kind="stable")
        counts = np.bincount(b, minlength=16)
        if np.any(counts > np.array(capc) * P):
            raise OverflowError(list(counts))
        s_loc = (s & (Q - 1)).astype(np.int16)
        d_loc = (d & (Q - 1)).astype(np.int16)
        sw_parts, dw_parts = [], []
        pos = 0
        for bi in range(16):
            es = order[pos: pos + counts[bi]]
            pos += counts[bi]
            cap = capc[bi] * P
            sw_parts.append(_wrap16(s_loc[es], cap))
            dw_parts.append(_wrap16(d_loc[es], cap))
        in_maps.append({
            "x": x,
            "src16": np.ascontiguousarray(np.concatenate(sw_parts, axis=1)),
            "dst16": np.ascontiguousarray(np.concatenate(dw_parts, axis=1)),
            "l_param": l,
        })
        orders.append(order)
        counts_all.append(counts)
    return in_maps, orders, counts_all


def _unshard(results, orders, counts_all, capc=CAPC):
    out = np.empty(E, np.float32)
    offs = np.cumsum([0] + [c for c in capc])
    for c in range(NUM_CORES):
        dev = results[c]["out"]            # [128, cols]
        order, counts = orders[c], counts_all[c]
        core_out = np.empty(EC, np.float32)
        pos = 0
        for bi in range(16):
            cnt = counts[bi]
            vals = dev[:, offs[bi]: offs[bi] + capc[bi]].T.ravel()[:cnt]
            core_out[order[pos: pos + cnt]] = vals
            pos += cnt
        out[c * EC:(c + 1) * EC] = core_out
    return out.reshape(E, 1)


def kernel(x, edge_label_index, l_param):
    capc = CAPC
    while True:
        try:
            in_maps, orders, counts = make_in_maps(
                x, edge_label_index, l_param, capc)
            break
        except OverflowError as e:
            # grow capacities to fit (rounded up to tile granularity)
            need = [max(int(np.ceil(n / P / 8)) * 8, c)
                    for n, c in zip(e.args[0], capc)]
            capc = tuple(need)
    nc = _get_compiled(capc)
    res = run_bass_kernel_spmd(nc, in_maps, list(range(NUM_CORES)))
    return _unshard(res.results, orders, counts, capc)
